# revision 4
# baseline (speedup 1.0000x reference)
"""Trainium2 Bass kernel for nn_CustomAttention (additive-tanh-score attention).

Math: out = softmax_m(mean_d tanh(q[n,d] + k[m,d])) @ v, with q = x1 Wq^T,
k = x2 Wk^T, v = x2 Wv^T.  The DropKey mask term (bernoulli * -1e-12) is below
fp32 resolution and is dropped.

Algorithm: tanh(s) is approximated by an odd-harmonic sine series
    tanh(s) ~= sum_i b_i sin(j_i * pi * s / L),   j_i = 1,3,...,19
so with theta_x = (pi/L) q_d, theta_y = (pi/L) k_d:
    sin(j(theta_x+theta_y)) = sin(j theta_x) cos(j theta_y)
                            + cos(j theta_x) sin(j theta_y)
which turns the [N,M,D] tanh reduction into a TensorE matmul with contraction
(2 * K * D).  Harmonic features sin/cos(j theta) are generated with the
three-term recurrence X_{j+2} = 2 cos(2 theta) X_j - X_{j-2} on the Vector
engine (ACT's Sin spline only covers [-pi, pi], so high harmonics cannot be
evaluated directly).  The series coefficients b_i are folded into the q-side
recurrence.  Softmax needs no max-subtraction (scores are means of tanh, so
|score| <= ~1) and the row-sum rides the output matmul as a ones-column of v.

Sharding: data-parallel over batch, 2 batches per core, 8 cores.
"""

import numpy as np

import concourse.bass as bass
import concourse.bacc as bacc
import concourse.mybir as mybir
from concourse.tile import TileContext
from concourse.bass_utils import run_bass_kernel_spmd

F32 = mybir.dt.float32
AF = mybir.ActivationFunctionType
OP = mybir.AluOpType

# ---- fitted odd-harmonic sine series for tanh on |s| <= 6.96, L = half period
L_FIT = 11.504294395446777
B_COEF = [1.2350389628018632, 0.3265108349460186, 0.12969070001050748,
          0.054376297113699686, 0.022998492809357177, 0.009767106371444135,
          0.00412679540803737, 0.0017537431901711064, 0.0007544607820725653,
          0.0002955722082474476]
K = len(B_COEF)          # number of odd harmonics (1, 3, ..., 2K-1)

NCORES = 8
B_TOT, N, D = 16, 512, 64
BPC = B_TOT // NCORES    # batches per core
W = BPC * N              # free width when both batches are packed
PI = float(np.pi)

_cache = {}


def _build():
    """Build + compile the per-core Bass program (identical on all cores)."""
    nc = bacc.Bacc("TRN2", target_bir_lowering=False, debug=False)

    x1_d = nc.dram_tensor("x1", [BPC, N, D], F32, kind="ExternalInput")
    x2_d = nc.dram_tensor("x2", [BPC, N, D], F32, kind="ExternalInput")
    wq2_d = nc.dram_tensor("wq2", [D, 128], F32, kind="ExternalInput")
    wk2_d = nc.dram_tensor("wk2", [D, 128], F32, kind="ExternalInput")
    wv_d = nc.dram_tensor("wv", [D, D], F32, kind="ExternalInput")
    id_d = nc.dram_tensor("ident", [128, 128], F32, kind="ExternalInput")
    bq_d = nc.dram_tensor("biasq", [128, 1], F32, kind="ExternalInput")
    bk_d = nc.dram_tensor("biask", [128, 1], F32, kind="ExternalInput")
    out_d = nc.dram_tensor("out", [BPC, N, D], F32, kind="ExternalOutput")

    with TileContext(nc) as tc:
        with (
            tc.tile_pool(name="const", bufs=1) as const,
            tc.tile_pool(name="xin", bufs=1) as xin,
            tc.tile_pool(name="xt", bufs=2) as xt,
            tc.tile_pool(name="th", bufs=1) as thp,
            tc.tile_pool(name="mul", bufs=2) as mulp,
            tc.tile_pool(name="sqp", bufs=2) as sqp,
            tc.tile_pool(name="ladq", bufs=4) as ladq,
            tc.tile_pool(name="ladk", bufs=4) as ladk,
            tc.tile_pool(name="tmpq", bufs=2) as tmpq,
            tc.tile_pool(name="tmpk", bufs=2) as tmpk,
            tc.tile_pool(name="vaug", bufs=2) as vaugp,
            tc.tile_pool(name="ep", bufs=8) as ep,
            tc.tile_pool(name="osb", bufs=2) as osb,
            tc.tile_pool(name="rp", bufs=8) as rp,
            tc.tile_pool(name="ps", bufs=8, space="PSUM") as ps,
        ):
            # ---------- constants ----------
            sb_wq2 = const.tile([D, 128], F32)
            nc.sync.dma_start(out=sb_wq2, in_=wq2_d[:, :])
            sb_wk2 = const.tile([D, 128], F32)
            nc.sync.dma_start(out=sb_wk2, in_=wk2_d[:, :])
            sb_wv = const.tile([D, D], F32)
            nc.sync.dma_start(out=sb_wv, in_=wv_d[:, :])
            sb_id = const.tile([128, 128], F32)
            nc.sync.dma_start(out=sb_id, in_=id_d[:, :])
            sb_bq = const.tile([128, 1], F32)
            nc.sync.dma_start(out=sb_bq, in_=bq_d[:, :])
            sb_bk = const.tile([128, 1], F32)
            nc.sync.dma_start(out=sb_bk, in_=bk_d[:, :])

            # ---------- inputs ----------
            sb_x1 = xin.tile([128, BPC, 4, D], F32)
            nc.sync.dma_start(
                out=sb_x1, in_=x1_d.ap().rearrange("b (a p) d -> p b a d", p=128))
            sb_x2 = xin.tile([128, BPC, 4, D], F32)
            nc.sync.dma_start(
                out=sb_x2, in_=x2_d.ap().rearrange("b (a p) d -> p b a d", p=128))

            # ---------- prologue: transposes, projections, v ----------
            sb_thq = thp.tile([128, W], F32)   # [sin-half d; cos-half d] x (b, n)
            sb_thk = thp.tile([128, W], F32)
            vaug = []
            for b in range(BPC):
                ps_x1t = ps.tile([D, N], F32, tag="bank")
                ps_x2t = ps.tile([D, N], F32, tag="bank")
                for a in range(4):
                    nc.tensor.transpose(
                        ps_x1t[:, a * 128:(a + 1) * 128], sb_x1[:, b, a, :], sb_id)
                    nc.tensor.transpose(
                        ps_x2t[:, a * 128:(a + 1) * 128], sb_x2[:, b, a, :], sb_id)
                sb_x1t = xt.tile([D, N], F32)
                nc.scalar.copy(sb_x1t, ps_x1t)
                sb_x2t = xt.tile([D, N], F32)
                nc.scalar.copy(sb_x2t, ps_x2t)

                ps_thq = ps.tile([128, N], F32, tag="bank")
                nc.tensor.matmul(ps_thq, sb_wq2, sb_x1t, start=True, stop=True)
                nc.scalar.copy(sb_thq[:, b * N:(b + 1) * N], ps_thq)
                ps_thk = ps.tile([128, N], F32, tag="bank")
                nc.tensor.matmul(ps_thk, sb_wk2, sb_x2t, start=True, stop=True)
                nc.scalar.copy(sb_thk[:, b * N:(b + 1) * N], ps_thk)

                ps_v = ps.tile([128, 4, D], F32, tag="bank")
                for a in range(4):
                    nc.tensor.matmul(
                        ps_v[:, a, :], sb_x2t[:, a * 128:(a + 1) * 128], sb_wv,
                        start=True, stop=True)
                sb_va = vaugp.tile([128, 4, D + 1], F32)
                nc.vector.memset(sb_va, 1.0)
                nc.scalar.copy(sb_va[:, :, 0:D], ps_v)
                vaug.append(sb_va)

            # ---------- harmonic bases ----------
            # q side: X_i = b-scaled [sin((2i+1)th); cos((2i+1)th)]
            # k side: Z_i =          [cos((2i+1)th); sin((2i+1)th)]
            x1b = ladq.tile([128, W], F32, tag="ladq")      # [sin th; cos th]
            nc.scalar.activation(x1b, sb_thq, AF.Sin, bias=sb_bq[:, 0:1], scale=1.0)
            xm1 = ladq.tile([128, W], F32, tag="ladq")      # j = -1: [-sin th; cos th]
            nc.scalar.activation(xm1, sb_thq, AF.Sin, bias=sb_bq[:, 0:1], scale=-1.0)
            z1 = ladk.tile([128, W], F32, tag="ladk")       # [cos th; sin th]
            nc.scalar.activation(z1, sb_thk, AF.Sin, bias=sb_bk[:, 0:1], scale=1.0)
            zm1 = ladk.tile([128, W], F32, tag="ladk")      # j = -1: [cos th; -sin th]
            nc.scalar.activation(zm1, sb_thk, AF.Sin, bias=sb_bk[:, 0:1], scale=-1.0)

            # multipliers cos(2 theta) = 1 - 2 sin^2(theta), both halves
            sb_m2 = []
            for th in (sb_thq, sb_thk):
                s1 = sqp.tile([128, W], F32, tag="sq", name="s1")
                nc.scalar.activation(s1, th, AF.Sin, bias=0.0, scale=1.0)
                sq = sqp.tile([128, W], F32, tag="sq", name="sq")
                nc.scalar.activation(sq, s1, AF.Square, bias=0.0, scale=1.0)
                m2 = mulp.tile([128, W], F32)
                nc.vector.tensor_scalar(m2, sq, -2.0, 1.0, OP.mult, OP.add)
                sb_m2.append(m2)
            m2q, m2k = sb_m2

            # scale q-side base by b_0
            xs1 = ladq.tile([128, W], F32, tag="ladq")
            nc.vector.tensor_scalar(xs1, x1b, float(B_COEF[0]), None, OP.mult)

            # ---------- scores psum ----------
            ps_sc = [[ps.tile([128, N], F32, tag="bank", name=f"ps_sc_{b}_{mt}")
                      for mt in range(4)] for b in range(BPC)]

            xq_prev, xq_cur = xm1, xs1
            zk_prev, zk_cur = zm1, z1
            for i in range(K):
                for b in range(BPC):
                    for mt in range(4):
                        nc.tensor.matmul(
                            ps_sc[b][mt],
                            zk_cur[:, b * N + mt * 128: b * N + (mt + 1) * 128],
                            xq_cur[:, b * N:(b + 1) * N],
                            start=(i == 0), stop=(i == K - 1))
                if i < K - 1:
                    # q side, b-folded:
                    #   tmp = (2 b_{i+1}/b_i) * X_i * cos2th
                    #   X_{i+1} = tmp - (b_{i+1}/b_{i-1}) * X_{i-1}
                    rm = 2.0 * B_COEF[i + 1] / B_COEF[i]
                    rs = B_COEF[i + 1] / (1.0 if i == 0 else B_COEF[i - 1])
                    tq = tmpq.tile([128, W], F32)
                    nc.vector.scalar_tensor_tensor(
                        tq, xq_cur, float(rm), m2q, OP.mult, OP.mult)
                    xq_new = ladq.tile([128, W], F32, tag="ladq", name="xq_new")
                    nc.vector.scalar_tensor_tensor(
                        xq_new, xq_prev, float(-rs), tq, OP.mult, OP.add)
                    xq_prev, xq_cur = xq_cur, xq_new
                    # k side, unscaled
                    tk = tmpk.tile([128, W], F32)
                    nc.vector.scalar_tensor_tensor(
                        tk, zk_cur, 2.0, m2k, OP.mult, OP.mult)
                    zk_new = ladk.tile([128, W], F32, tag="ladk", name="zk_new")
                    nc.vector.scalar_tensor_tensor(
                        zk_new, zk_prev, -1.0, tk, OP.mult, OP.add)
                    zk_prev, zk_cur = zk_cur, zk_new

            # ---------- epilogue: softmax (no max-sub) + output ----------
            for b in range(BPC):
                e_tiles = []
                for mt in range(4):
                    e = ep.tile([128, N], F32)
                    nc.scalar.activation(
                        e, ps_sc[b][mt], AF.Exp, bias=0.0, scale=1.0 / D)
                    e_tiles.append(e)
                ps_ot = ps.tile([D + 1, N], F32, tag="bank")
                for mt in range(4):
                    nc.tensor.matmul(
                        ps_ot, vaug[b][:, mt, :], e_tiles[mt],
                        start=(mt == 0), stop=(mt == 3))
                sb_ot = osb.tile([D + 1, N], F32)
                nc.scalar.copy(sb_ot, ps_ot)
                ps_tr = ps.tile([128, 4, D + 1], F32, tag="bank")
                for a in range(4):
                    nc.tensor.transpose(
                        ps_tr[:, a, :], sb_ot[:, a * 128:(a + 1) * 128],
                        sb_id[0:D + 1, 0:D + 1])
                o_sb = osb.tile([128, 4, D], F32)
                for a in range(4):
                    r = rp.tile([128, 1], F32)
                    nc.vector.reciprocal(r, ps_tr[:, a, D:D + 1])
                    nc.vector.tensor_scalar(
                        o_sb[:, a, :], ps_tr[:, a, 0:D], r[:, 0:1], None, OP.mult)
                nc.sync.dma_start(
                    out=out_d.ap().rearrange("b (a p) d -> p b a d", p=128)[:, b],
                    in_=o_sb)

    nc.compile()
    return nc


def _host_prep(Wq, Wk, Wv):
    scale = np.float32(np.pi / L_FIT)
    wq2 = np.concatenate([(scale * Wq).T, (scale * Wq).T], axis=1).astype(np.float32)
    wk2 = np.concatenate([(scale * Wk).T, (scale * Wk).T], axis=1).astype(np.float32)
    wv = np.ascontiguousarray(Wv.T.astype(np.float32))
    ident = np.eye(128, dtype=np.float32)
    biasq = np.concatenate([np.zeros(64), np.full(64, np.pi / 2)]).astype(
        np.float32).reshape(128, 1)
    biask = np.concatenate([np.full(64, np.pi / 2), np.zeros(64)]).astype(
        np.float32).reshape(128, 1)
    return wq2, wk2, wv, ident, biasq, biask


def kernel(input1, input2, Wq, Wk, Wv):
    if "nc" not in _cache:
        _cache["nc"] = _build()
    nc = _cache["nc"]

    wq2, wk2, wv, ident, biasq, biask = _host_prep(
        np.asarray(Wq), np.asarray(Wk), np.asarray(Wv))
    x1 = np.ascontiguousarray(np.asarray(input1, dtype=np.float32))
    x2 = np.ascontiguousarray(np.asarray(input2, dtype=np.float32))

    in_maps = []
    for c in range(NCORES):
        in_maps.append({
            "x1": x1[c * BPC:(c + 1) * BPC],
            "x2": x2[c * BPC:(c + 1) * BPC],
            "wq2": wq2, "wk2": wk2, "wv": wv,
            "ident": ident, "biasq": biasq, "biask": biask,
        })
    res = run_bass_kernel_spmd(nc, in_maps, core_ids=list(range(NCORES)))
    out = np.concatenate([res.results[c]["out"] for c in range(NCORES)], axis=0)
    return out.astype(np.float32)


# revision 7
# speedup vs baseline: 1.4475x; 1.4475x over previous
"""Trainium2 Bass kernel for nn_CustomAttention (additive-tanh-score attention).

Math: out = softmax_m(mean_d tanh(q[n,d] + k[m,d])) @ v, with q = x1 Wq^T,
k = x2 Wk^T, v = x2 Wv^T.  The DropKey mask term (bernoulli * -1e-12) is below
fp32 resolution and is dropped.

Algorithm: tanh(s) is approximated by an odd-harmonic sine series
    tanh(s) ~= sum_i b_i sin(j_i * pi * s / L),   j_i = 1,3,...,19
so with theta_x = (pi/L) q_d, theta_y = (pi/L) k_d:
    sin(j(theta_x+theta_y)) = sin(j theta_x) cos(j theta_y)
                            + cos(j theta_x) sin(j theta_y)
which turns the [N,M,D] tanh reduction into a TensorE matmul with contraction
(2 * K * D).  Harmonic features sin/cos(j theta) are generated with the
three-term recurrence X_{j+2} = 2 cos(2 theta) X_j - X_{j-2} on the Vector
engine (ACT's Sin spline only covers [-pi, pi], so high harmonics cannot be
evaluated directly).  The series coefficients b_i are folded into the q-side
recurrence.  Softmax needs no max-subtraction (scores are means of tanh, so
|score| <= ~1) and the row-sum rides the output matmul as a ones-column of v.

Sharding: data-parallel over batch, 2 batches per core, 8 cores.
"""

import numpy as np

import concourse.bass as bass
import concourse.bacc as bacc
import concourse.mybir as mybir
from concourse.tile import TileContext
from concourse.bass_utils import run_bass_kernel_spmd

F32 = mybir.dt.float32
F32R = mybir.dt.float32r
AF = mybir.ActivationFunctionType
OP = mybir.AluOpType

# ---- fitted odd-harmonic sine series for tanh on |s| <= 6.96, L = half period
L_FIT = 11.504294395446777
B_COEF = [1.2350389628018632, 0.3265108349460186, 0.12969070001050748,
          0.054376297113699686, 0.022998492809357177, 0.009767106371444135,
          0.00412679540803737, 0.0017537431901711064, 0.0007544607820725653,
          0.0002955722082474476]
K = len(B_COEF)          # number of odd harmonics (1, 3, ..., 2K-1)

NCORES = 8
B_TOT, N, D = 16, 512, 64
BPC = B_TOT // NCORES    # batches per core
W = BPC * N              # free width when both batches are packed
PI = float(np.pi)

_cache = {}


def _build():
    """Build + compile the per-core Bass program (identical on all cores)."""
    nc = bacc.Bacc("TRN2", target_bir_lowering=False, debug=False)

    x1_d = nc.dram_tensor("x1", [BPC, N, D], F32, kind="ExternalInput")
    x2_d = nc.dram_tensor("x2", [BPC, N, D], F32, kind="ExternalInput")
    wq2_d = nc.dram_tensor("wq2", [D, 128], F32, kind="ExternalInput")
    wk2_d = nc.dram_tensor("wk2", [D, 128], F32, kind="ExternalInput")
    wv_d = nc.dram_tensor("wv", [D, D], F32, kind="ExternalInput")
    id_d = nc.dram_tensor("ident", [128, 128], F32, kind="ExternalInput")
    bq_d = nc.dram_tensor("biasq", [128, 1], F32, kind="ExternalInput")
    bk_d = nc.dram_tensor("biask", [128, 1], F32, kind="ExternalInput")
    out_d = nc.dram_tensor("out", [BPC, N, D], F32, kind="ExternalOutput")

    with TileContext(nc) as tc:
        with (
            tc.tile_pool(name="const", bufs=1) as const,
            tc.tile_pool(name="xin", bufs=1) as xin,
            tc.tile_pool(name="xt", bufs=2) as xt,
            tc.tile_pool(name="th", bufs=1) as thp,
            tc.tile_pool(name="mul", bufs=2) as mulp,
            tc.tile_pool(name="sqp", bufs=2) as sqp,
            tc.tile_pool(name="ladq", bufs=4) as ladq,
            tc.tile_pool(name="ladk", bufs=4) as ladk,
            tc.tile_pool(name="tmpq", bufs=2) as tmpq,
            tc.tile_pool(name="tmpk", bufs=2) as tmpk,
            tc.tile_pool(name="vaug", bufs=2) as vaugp,
            tc.tile_pool(name="ep", bufs=8) as ep,
            tc.tile_pool(name="osb", bufs=2) as osb,
            tc.tile_pool(name="rp", bufs=8) as rp,
            tc.tile_pool(name="ps", bufs=8, space="PSUM") as ps,
        ):
            # ---------- constants ----------
            sb_wq2 = const.tile([D, 128], F32)
            nc.sync.dma_start(out=sb_wq2, in_=wq2_d[:, :])
            sb_wk2 = const.tile([D, 128], F32)
            nc.sync.dma_start(out=sb_wk2, in_=wk2_d[:, :])
            sb_wv = const.tile([D, D], F32)
            nc.sync.dma_start(out=sb_wv, in_=wv_d[:, :])
            sb_id = const.tile([128, 128], F32)
            nc.sync.dma_start(out=sb_id, in_=id_d[:, :])
            sb_bq = const.tile([128, 1], F32)
            nc.sync.dma_start(out=sb_bq, in_=bq_d[:, :])
            sb_bk = const.tile([128, 1], F32)
            nc.sync.dma_start(out=sb_bk, in_=bk_d[:, :])

            # ---------- inputs ----------
            sb_x1 = xin.tile([128, BPC, 4, D], F32)
            nc.sync.dma_start(
                out=sb_x1, in_=x1_d.ap().rearrange("b (a p) d -> p b a d", p=128))
            sb_x2 = xin.tile([128, BPC, 4, D], F32)
            nc.sync.dma_start(
                out=sb_x2, in_=x2_d.ap().rearrange("b (a p) d -> p b a d", p=128))

            # ---------- prologue: transposes, projections, v ----------
            sb_thq = thp.tile([128, W], F32)   # [sin-half d; cos-half d] x (b, n)
            sb_thk = thp.tile([128, W], F32)
            vaug = []
            for b in range(BPC):
                ps_x1t = ps.tile([D, N], F32, tag="bank")
                ps_x2t = ps.tile([D, N], F32, tag="bank")
                for a in range(4):
                    nc.tensor.transpose(
                        ps_x1t[:, a * 128:(a + 1) * 128], sb_x1[:, b, a, :], sb_id)
                    nc.tensor.transpose(
                        ps_x2t[:, a * 128:(a + 1) * 128], sb_x2[:, b, a, :], sb_id)
                sb_x1t = xt.tile([D, N], F32)
                nc.scalar.copy(sb_x1t, ps_x1t)
                sb_x2t = xt.tile([D, N], F32)
                nc.scalar.copy(sb_x2t, ps_x2t)

                ps_thq = ps.tile([128, N], F32, tag="bank")
                nc.tensor.matmul(ps_thq, sb_wq2, sb_x1t, start=True, stop=True)
                nc.scalar.copy(sb_thq[:, b * N:(b + 1) * N], ps_thq)
                ps_thk = ps.tile([128, N], F32, tag="bank")
                nc.tensor.matmul(ps_thk, sb_wk2, sb_x2t, start=True, stop=True)
                nc.scalar.copy(sb_thk[:, b * N:(b + 1) * N], ps_thk)

                ps_v = ps.tile([128, 4, D], F32, tag="bank")
                for a in range(4):
                    nc.tensor.matmul(
                        ps_v[:, a, :], sb_x2t[:, a * 128:(a + 1) * 128], sb_wv,
                        start=True, stop=True)
                sb_va = vaugp.tile([128, 4, D + 1], F32R)
                nc.vector.memset(sb_va.bitcast(F32), 1.0)
                nc.scalar.copy(sb_va[:, :, 0:D], ps_v)
                vaug.append(sb_va)

            # ---------- harmonic bases ----------
            # q side: X_i = b-scaled [sin((2i+1)th); cos((2i+1)th)]
            # k side: Z_i =          [cos((2i+1)th); sin((2i+1)th)]
            x1b = ladq.tile([128, W], F32, tag="ladq")      # [sin th; cos th]
            nc.scalar.activation(x1b, sb_thq, AF.Sin, bias=sb_bq[:, 0:1], scale=1.0)
            xm1 = ladq.tile([128, W], F32, tag="ladq")      # j = -1: [-sin th; cos th]
            nc.scalar.activation(xm1, sb_thq, AF.Sin, bias=sb_bq[:, 0:1], scale=-1.0)
            z1 = ladk.tile([128, W], F32R, tag="ladk")       # [cos th; sin th]
            nc.scalar.activation(z1, sb_thk, AF.Sin, bias=sb_bk[:, 0:1], scale=1.0)
            zm1 = ladk.tile([128, W], F32, tag="ladk")      # j = -1: [cos th; -sin th]
            nc.scalar.activation(zm1, sb_thk, AF.Sin, bias=sb_bk[:, 0:1], scale=-1.0)

            # multipliers cos(2 theta) = 1 - 2 sin^2(theta), both halves
            sb_m2 = []
            for th in (sb_thq, sb_thk):
                s1 = sqp.tile([128, W], F32, tag="sq", name="s1")
                nc.scalar.activation(s1, th, AF.Sin, bias=0.0, scale=1.0)
                sq = sqp.tile([128, W], F32, tag="sq", name="sq")
                nc.scalar.activation(sq, s1, AF.Square, bias=0.0, scale=1.0)
                m2 = mulp.tile([128, W], F32)
                nc.vector.tensor_scalar(m2, sq, -2.0, 1.0, OP.mult, OP.add)
                sb_m2.append(m2)
            m2q, m2k = sb_m2

            # scale q-side base by b_0
            xs1 = ladq.tile([128, W], F32R, tag="ladq")
            nc.vector.tensor_scalar(xs1, x1b, float(B_COEF[0]), None, OP.mult)

            # ---------- scores psum ----------
            ps_sc = [[ps.tile([128, N], F32, tag="bank", name=f"ps_sc_{b}_{mt}")
                      for mt in range(4)] for b in range(BPC)]

            xq_prev, xq_cur = xm1, xs1
            zk_prev, zk_cur = zm1, z1
            for i in range(K):
                for b in range(BPC):
                    for mt in range(4):
                        nc.tensor.matmul(
                            ps_sc[b][mt],
                            zk_cur[:, b * N + mt * 128: b * N + (mt + 1) * 128],
                            xq_cur[:, b * N:(b + 1) * N],
                            start=(i == 0), stop=(i == K - 1))
                if i < K - 1:
                    # q side, b-folded:
                    #   tmp = (2 b_{i+1}/b_i) * X_i * cos2th
                    #   X_{i+1} = tmp - (b_{i+1}/b_{i-1}) * X_{i-1}
                    rm = 2.0 * B_COEF[i + 1] / B_COEF[i]
                    rs = B_COEF[i + 1] / (1.0 if i == 0 else B_COEF[i - 1])
                    tq = tmpq.tile([128, W], F32)
                    nc.vector.scalar_tensor_tensor(
                        tq, xq_cur, float(rm), m2q, OP.mult, OP.mult)
                    xq_new = ladq.tile([128, W], F32R, tag="ladq", name="xq_new")
                    nc.vector.scalar_tensor_tensor(
                        xq_new, xq_prev, float(-rs), tq, OP.mult, OP.add)
                    xq_prev, xq_cur = xq_cur, xq_new
                    # k side, unscaled
                    tk = tmpk.tile([128, W], F32)
                    nc.vector.scalar_tensor_tensor(
                        tk, zk_cur, 2.0, m2k, OP.mult, OP.mult)
                    zk_new = ladk.tile([128, W], F32R, tag="ladk", name="zk_new")
                    nc.vector.scalar_tensor_tensor(
                        zk_new, zk_prev, -1.0, tk, OP.mult, OP.add)
                    zk_prev, zk_cur = zk_cur, zk_new

            # ---------- epilogue: softmax (no max-sub) + output ----------
            for b in range(BPC):
                e_tiles = []
                for mt in range(4):
                    e = ep.tile([128, N], F32R)
                    nc.scalar.activation(
                        e, ps_sc[b][mt], AF.Exp, bias=0.0, scale=1.0 / D)
                    e_tiles.append(e)
                ps_ot = ps.tile([D + 1, N], F32, tag="bank")
                for mt in range(4):
                    nc.tensor.matmul(
                        ps_ot, vaug[b][:, mt, :], e_tiles[mt],
                        start=(mt == 0), stop=(mt == 3))
                sb_ot = osb.tile([D + 1, N], F32)
                nc.scalar.copy(sb_ot, ps_ot)
                ps_tr = ps.tile([128, 4, D + 1], F32, tag="bank")
                for a in range(4):
                    nc.tensor.transpose(
                        ps_tr[:, a, :], sb_ot[:, a * 128:(a + 1) * 128],
                        sb_id[0:D + 1, 0:D + 1])
                o_sb = osb.tile([128, 4, D], F32)
                for a in range(4):
                    r = rp.tile([128, 1], F32)
                    nc.vector.reciprocal(r, ps_tr[:, a, D:D + 1])
                    nc.vector.tensor_scalar(
                        o_sb[:, a, :], ps_tr[:, a, 0:D], r[:, 0:1], None, OP.mult)
                nc.sync.dma_start(
                    out=out_d.ap().rearrange("b (a p) d -> p b a d", p=128)[:, b],
                    in_=o_sb)

    nc.compile()
    return nc


def _host_prep(Wq, Wk, Wv):
    scale = np.float32(np.pi / L_FIT)
    wq2 = np.concatenate([(scale * Wq).T, (scale * Wq).T], axis=1).astype(np.float32)
    wk2 = np.concatenate([(scale * Wk).T, (scale * Wk).T], axis=1).astype(np.float32)
    wv = np.ascontiguousarray(Wv.T.astype(np.float32))
    ident = np.eye(128, dtype=np.float32)
    biasq = np.concatenate([np.zeros(64), np.full(64, np.pi / 2)]).astype(
        np.float32).reshape(128, 1)
    biask = np.concatenate([np.full(64, np.pi / 2), np.zeros(64)]).astype(
        np.float32).reshape(128, 1)
    return wq2, wk2, wv, ident, biasq, biask


def kernel(input1, input2, Wq, Wk, Wv):
    if "nc" not in _cache:
        _cache["nc"] = _build()
    nc = _cache["nc"]

    wq2, wk2, wv, ident, biasq, biask = _host_prep(
        np.asarray(Wq), np.asarray(Wk), np.asarray(Wv))
    x1 = np.ascontiguousarray(np.asarray(input1, dtype=np.float32))
    x2 = np.ascontiguousarray(np.asarray(input2, dtype=np.float32))

    in_maps = []
    for c in range(NCORES):
        in_maps.append({
            "x1": x1[c * BPC:(c + 1) * BPC],
            "x2": x2[c * BPC:(c + 1) * BPC],
            "wq2": wq2, "wk2": wk2, "wv": wv,
            "ident": ident, "biasq": biasq, "biask": biask,
        })
    res = run_bass_kernel_spmd(nc, in_maps, core_ids=list(range(NCORES)))
    out = np.concatenate([res.results[c]["out"] for c in range(NCORES)], axis=0)
    return out.astype(np.float32)
